# revision 5
# baseline (speedup 1.0000x reference)
"""Multi-head attention kernel for Trainium2, one head per NeuronCore.

Math per head h, batch b (n = 2304 tokens, c = 256, d = 32):
  q,k = W_{q,k} @ x   (joint matmul, bf16), v = x^T @ Wv^T (transposed)
  S''[j,i] = A16*(k_j . q_i * scale + B[j,i])   computed in fp8 DoubleRow
    matmuls with the A16 = 128/ln2 Schraudolph scale folded into the
    host-prepped weights and bias:
      m2 = {k_lo, q_hi} + {I, B''}   (DR pair, K=128: identity adds bias)
      m1 = {k_hi, q_hi} + {k_hi, q_lo}  (DR pair, K=32, stride-0 lhsT)
    where q = q_hi + q_lo and k = k_hi + k_lo are exact fp8e4m3 hi/lo
    splits, so S'' carries only the tiny -k_lo*q_lo residual.
  P[j,i] (bf16) = exp(S''/A16) via either:
      'A': Act exp with scale=1/A16   (exact)
      'D': DVE Schraudolph: bf16bits(P) = int16(round(S'' + B16))
  O accumulation: v_ext = [v.T | 1] per 128-token chunk, col-tiled 2-way
    (even chunks -> psum rows 0..32, odd -> 64..96).
  out_un[c,i] = sum_r wo2[c,r] * o_t[r,i]  (f32r, wo2 = [wo;0;wo;0])
Host: out = sum_h out_un_h / (sums0_h + sums1_h) + b_out.
"""

import sys

for _p in ("/opt/trn_rl_repo", "/root/.axon_site/_ro/trn_rl_repo"):
    if _p not in sys.path:
        sys.path.append(_p)

import numpy as np
import ml_dtypes

import concourse.bacc as bacc
import concourse.mybir as mybir
import concourse.tile as tile
from concourse import bass_utils
from concourse.ap import AP

HEADS = 8
D = 32
SCALE = D ** -0.5
B = 4
C = 256
N = 2304
H = W = 48
NJ = 18                     # 128-row j-chunks
NG = 9                      # pair-groups per (b, i-block)
IBLOCKS = [(0, 512), (512, 512), (1024, 512), (1536, 512), (2048, 256)]

F32 = mybir.dt.float32
F32R = mybir.dt.float32r
BF16 = mybir.dt.bfloat16
I16 = mybir.dt.int16
FP8 = mybir.dt.float8e4
E4NP = ml_dtypes.float8_e4m3fn
EXP = mybir.ActivationFunctionType.Exp
ADD = mybir.AluOpType.add
MAX = mybir.AluOpType.max
DR = mybir.MatmulPerfMode.DoubleRow

A16 = float(128.0 / np.log(2.0))     # Schraudolph scale folded into wq/wk/B
SCH_B16 = 16256.0 - 7.37             # bf16 exp bias, mean-centered (RN conv)

# k-region free-layout (fp8): per chunk c: k_hi at 256c, k_lo at 256c+128
# (rows 0..31, rows 32..127 zero); identity I at KREG_I (rows 0..127).
KREG_I = NJ * 256            # 4608
KREG_W = KREG_I + 128        # 4736
# mega rhs tile (per i-block) free-layout (fp8), slot width 512:
# q_hi(b) at b*512, q_lo(b) at (4+b)*512, B-chunk(c) at (8+c)*512.
# q rows 0..31 data, 32..127 zero.  Slot strides stay under the 16-bit
# ISA step_elem limit.
SLOT = 512
MEGA_W = (8 + NJ) * SLOT

# tuning knobs
ASSIGN = (
    "ADADADADA",
    "ADADADADA",
    "ADADADADA",
    "ADADADADA",
    "ADADADADA",
)
LAG_E = 2                  # pair-groups of slack: S psum -> exp
LAG_O = 8                  # exp -> O matmuls
LAG_C = LAG_O              # unit closing (o_t evac + sums)
LAG_P = LAG_O + 4          # out-projection
PP_BUFS = 14
SPSUM_BUFS = 3
OUT_EVAC_ENG = "scalar"
QHI_EVAC_ENG = "scalar"
QLO_EVAC_ENG = "vector"


def _mk_ap(base_ap, extra_off, dims):
    return AP(base_ap.tensor, base_ap.offset + extra_off, dims)


def _emit(nc, reps=1):
    x_d = nc.dram_tensor("x", [B, C, N], BF16, kind="ExternalInput")
    wqk_d = nc.dram_tensor("wqk", [C, 64], BF16, kind="ExternalInput")
    wv_d = nc.dram_tensor("wv", [C, D], BF16, kind="ExternalInput")
    wo_d = nc.dram_tensor("wo", [97, C], F32, kind="ExternalInput")
    bt_d = nc.dram_tensor("bt", [N, N], FP8, kind="ExternalInput")
    id_d = nc.dram_tensor("ident", [128, 128], FP8, kind="ExternalInput")
    out_d = nc.dram_tensor("out_un", [B, C, N], F32, kind="ExternalOutput")
    sums_d = nc.dram_tensor("sums", [B, 2, N], F32, kind="ExternalOutput")

    with tile.TileContext(nc) as tc:
        with (
            tc.tile_pool(name="wpool", bufs=1) as wpool,
            tc.tile_pool(name="big", bufs=2) as bigpool,
            tc.tile_pool(name="kreg", bufs=4) as kpool,
            tc.tile_pool(name="ktmp", bufs=2) as ktpool,
            tc.tile_pool(name="mega", bufs=2) as megapool,
            tc.tile_pool(name="qst", bufs=4) as qstpool,
            tc.tile_pool(name="vext", bufs=4) as vpool,
            tc.tile_pool(name="pp", bufs=PP_BUFS) as ppool,
            tc.tile_pool(name="outsb", bufs=3) as outpool,
            tc.tile_pool(name="osb", bufs=2) as opool,
            tc.tile_pool(name="spsum", bufs=SPSUM_BUFS, space="PSUM") as spsum,
            tc.tile_pool(name="psO", bufs=1, space="PSUM") as psO,
            tc.tile_pool(name="psP", bufs=1, space="PSUM") as psP,
        ):
            # ---- static weights ----
            wqk_r = wpool.tile([128, 2, 64], BF16, tag="wqk")
            nc.sync.dma_start(wqk_r, wqk_d.ap().rearrange("(cc p) m -> p cc m", p=128))
            wv_r = wpool.tile([128, 2, D], BF16, tag="wv")
            nc.sync.dma_start(wv_r, wv_d.ap().rearrange("(cc p) m -> p cc m", p=128))
            wo_raw = wpool.tile([97, C], F32, tag="woraw")
            nc.sync.dma_start(wo_raw, wo_d.ap())
            wo_r = wpool.tile([97, C], F32R, tag="wor")
            nc.vector.tensor_copy(wo_r, wo_raw)

            # mega rhs tile: q slots + B chunks; zero q-slot pad rows once
            # pre-zero the q-slot pad rows of both rotating mega buffers
            for _ in range(2):
                mz = megapool.tile([128, MEGA_W], FP8, tag="mega")
                nc.vector.memset(mz[32:64, 0 : 8 * SLOT], 0.0)
                nc.vector.memset(mz[64:128, 0 : 8 * SLOT], 0.0)
            # k-region per batch: zero pad rows once, load identity
            kregs = []
            for b in range(B):
                kr = kpool.tile([128, KREG_W], FP8, tag="kreg")
                kregs.append(kr)
                nc.vector.memset(kr[32:64, 0:KREG_I], 0.0)
                nc.vector.memset(kr[64:128, 0:KREG_I], 0.0)
                nc.sync.dma_start(kr[:, KREG_I:KREG_W], id_d.ap())

            v_sb = [None] * B
            q_st = [None] * B

            def load_bias(ib):
                i0, iw = IBLOCKS[ib]
                mega = megapool.tile([128, MEGA_W], FP8, tag="mega")
                for c in range(NJ):
                    nc.sync.dma_start(
                        mega[:, (8 + c) * SLOT : (8 + c) * SLOT + iw],
                        bt_d.ap().rearrange("(c p) i -> p c i", p=128)[
                            :, c, i0 : i0 + iw
                        ],
                    )
                return mega

            def proj_batch(b):
                x_r = bigpool.tile([128, 2, N], BF16, tag="big")
                x_view = x_d.ap()[b].rearrange("(cc p) n -> p cc n", p=128)
                for cc in range(2):
                    nc.sync.dma_start(x_r[:, cc, :], x_view[:, cc, :])

                ktmp = ktpool.tile([64, KREG_I], FP8, tag="ktmp")
                qst = qstpool.tile([32, 2 * N], FP8, tag="qst")
                q_st[b] = qst
                qhi_eng = getattr(nc, QHI_EVAC_ENG)
                qlo_eng = getattr(nc, QLO_EVAC_ENG)
                for ib, (i0, iw) in enumerate(IBLOCKS):
                    pt = spsum.tile([128, 1024], F32, tag="sg")
                    for cc in range(2):
                        nc.tensor.matmul(
                            pt[0:64, 0:iw],
                            wqk_r[:, cc, :],
                            x_r[:, cc, i0 : i0 + iw],
                            start=(cc == 0),
                            stop=(cc == 1),
                        )
                    # evacs: q rows 0..31 -> staging; k rows 32..63 -> ktmp
                    qh_dst = qst[:, i0 : i0 + iw]
                    ql_dst = qst[:, N + i0 : N + i0 + iw]
                    if qhi_eng is nc.scalar:
                        qhi_eng.copy(qh_dst, pt[0:32, 0:iw])
                    else:
                        qhi_eng.tensor_copy(qh_dst, pt[0:32, 0:iw])
                    qlo_eng.tensor_sub(ql_dst, pt[0:32, 0:iw], qh_dst)
                    # k chunk slots covered by this i-block
                    nc4 = iw // 128
                    kh_dst = _mk_ap(
                        ktmp[32:64, 0:1], i0 * 2,
                        [list(ktmp[32:64, 0:1].ap[0]), [256, nc4], [1, 128]],
                    )
                    kl_dst = _mk_ap(
                        ktmp[32:64, 0:1], i0 * 2 + 128,
                        [list(ktmp[32:64, 0:1].ap[0]), [256, nc4], [1, 128]],
                    )
                    kh_src = pt[32:64, 0:iw].rearrange("p (c j) -> p c j", j=128)
                    if qhi_eng is nc.scalar:
                        qhi_eng.copy(kh_dst, kh_src)
                    else:
                        qhi_eng.tensor_copy(kh_dst, kh_src)
                    qlo_eng.tensor_sub(kl_dst, kh_src, kh_dst)
                # partition-shift k rows 32..63 -> kreg rows 0..31
                nc.sync.dma_start(kregs[b][0:32, 0:KREG_I], ktmp[32:64, :])

                # v transposed: v_T[j, d] per chunk, with ones column
                vext = vpool.tile([128, NJ * (D + 1)], BF16, tag="vext")
                v_sb[b] = vext
                nc.vector.memset(vext, 1.0)
                vt = spsum.tile([128, 1024], F32, tag="sg")
                for jc in range(NJ):
                    for cc in range(2):
                        nc.tensor.matmul(
                            vt[:, jc * D : (jc + 1) * D],
                            x_r[:, cc, jc * 128 : (jc + 1) * 128],
                            wv_r[:, cc, :],
                            start=(cc == 0),
                            stop=(cc == 1),
                        )
                nc.vector.tensor_copy(
                    vext.rearrange("p (jc m) -> p jc m", m=D + 1)[:, :, 0:D],
                    vt.rearrange("p (jc m) -> p jc m", m=D)[:, 0:NJ, :],
                )

            # Deferred-emission queues (see baseline): strict FIFO engine
            # queues mean a dependent instruction emitted too early
            # head-of-line-blocks its engine.  Stage queues with lags:
            #   e_queue: exp ops (wait on S psum)
            #   o_queue: O matmuls (wait on P)
            #   c_queue: per-(b,ib) o_t evac + sums DMA
            #   p_queue: per-(b,ib) out-projection
            e_queue, o_queue, c_queue, p_queue = [], [], [], []
            gctr = [0]
            lagged = ((e_queue, LAG_E), (o_queue, LAG_O),
                      (c_queue, LAG_C), (p_queue, LAG_P))

            def pump(cur):
                for q, lag in lagged:
                    while q and q[0][0] <= cur - lag:
                        q.pop(0)[1]()

            def flush_all():
                while any(q for q, _ in lagged):
                    cands = [
                        (q[0][0], i, q) for i, (q, _) in enumerate(lagged) if q
                    ]
                    cands.sort()
                    cands[0][2].pop(0)[1]()

            def attn(b, ib, mega, eb_path):
                i0, iw = IBLOCKS[ib]
                kreg = kregs[b]
                # stage this batch's q hi/lo into the i-block mega slots
                nc.sync.dma_start(
                    mega[0:32, b * SLOT : b * SLOT + iw],
                    q_st[b][:, i0 : i0 + iw],
                )
                nc.sync.dma_start(
                    mega[0:32, (4 + b) * SLOT : (4 + b) * SLOT + iw],
                    q_st[b][:, N + i0 : N + i0 + iw],
                )
                o_ps_box = []

                def get_o_ps():
                    if not o_ps_box:
                        o_ps_box.append(
                            psO.tile([128, 512], F32, tag="po", name="o_ps")
                        )
                    return o_ps_box[0]

                for g in range(NG):
                    path = eb_path[g]
                    s_ps = spsum.tile([128, 1024], F32, tag="sg")
                    for cz in range(2):
                        c = 2 * g + cz
                        off = cz * iw
                        # m2: {k_lo, q_hi} + {I, B''}  (K=128 DR)
                        lhsT2 = _mk_ap(
                            kreg[:, 0:1], 256 * c + 128,
                            [list(kreg[:, 0:1].ap[0]),
                             [KREG_I - (256 * c + 128), 2], [1, 128]],
                        )
                        rhs2 = _mk_ap(
                            mega[:, 0:1], b * SLOT,
                            [list(mega[:, 0:1].ap[0]),
                             [(8 + c - b) * SLOT, 2], [1, iw]],
                        )
                        nc.tensor.matmul(
                            s_ps[:, off : off + iw], lhsT2, rhs2,
                            start=True, stop=False, perf_mode=DR,
                        )
                        # m1: {k_hi, q_hi} + {k_hi, q_lo}  (K=32 DR)
                        lhsT1 = _mk_ap(
                            kreg[0:32, 0:1], 256 * c,
                            [list(kreg[0:32, 0:1].ap[0]), [0, 2], [1, 128]],
                        )
                        rhs1 = _mk_ap(
                            mega[0:32, 0:1], b * SLOT,
                            [list(mega[0:32, 0:1].ap[0]),
                             [4 * SLOT, 2], [1, iw]],
                        )
                        nc.tensor.matmul(
                            s_ps[:, off : off + iw], lhsT1, rhs1,
                            start=False, stop=True, perf_mode=DR,
                        )

                    p_t = ppool.tile([128, 1024], BF16, tag="pt")
                    gc = gctr[0]
                    gctr[0] += 1

                    def e_thunk(path=path, p_t=p_t, s_ps=s_ps, iw=iw):
                        if path == "A":
                            nc.scalar.activation(
                                p_t[:, 0 : 2 * iw], s_ps[:, 0 : 2 * iw], EXP,
                                bias=0.0, scale=float(1.0 / A16),
                            )
                        else:
                            nc.vector.tensor_scalar(
                                p_t.bitcast(I16)[:, 0 : 2 * iw],
                                s_ps[:, 0 : 2 * iw],
                                float(SCH_B16), 0.0, ADD, MAX,
                            )

                    e_queue.append((gc, e_thunk))

                    def o_thunk(g=g, p_t=p_t, b=b, iw=iw):
                        o_ps = get_o_ps()
                        for cz in range(2):
                            c = 2 * g + cz
                            base = 64 * (c % 2)
                            nc.tensor.matmul(
                                o_ps[base : base + D + 1, 0:iw],
                                v_sb[b][:, c * (D + 1) : (c + 1) * (D + 1)],
                                p_t[:, cz * iw : (cz + 1) * iw],
                                start=(c < 2),
                                stop=(c >= NJ - 2),
                            )

                    o_queue.append((gc, o_thunk))
                    pump(gc)

                def closing(b=b, i0=i0, iw=iw, gc_unit=gctr[0] - 1):
                    o_ps = get_o_ps()
                    o_t = opool.tile([128, 512], F32R, tag="ot")
                    # single evac covers both col-tile bands; rows 33..63
                    # carry stale psum but wo2 zeros there kill them.
                    nc.scalar.copy(o_t[0:97, 0:iw], o_ps[0:97, 0:iw])
                    nc.sync.dma_start(
                        sums_d.ap()[b, 0, i0 : i0 + iw],
                        o_t[D : D + 1, 0:iw].bitcast(F32),
                    )
                    nc.sync.dma_start(
                        sums_d.ap()[b, 1, i0 : i0 + iw],
                        o_t[96:97, 0:iw].bitcast(F32),
                    )

                    def outproj(b=b, i0=i0, iw=iw, o_t=o_t):
                        out_view = out_d.ap()[b].rearrange(
                            "(cc p) n -> p cc n", p=128
                        )
                        for cc in range(2):
                            op_ps = psP.tile(
                                [128, 512], F32, tag="pp", name="op_ps"
                            )
                            nc.tensor.matmul(
                                op_ps[:, 0:iw],
                                wo_r[0:97, cc * 128 : (cc + 1) * 128],
                                o_t[0:97, 0:iw],
                                start=True,
                                stop=True,
                            )
                            ev = outpool.tile([128, 512], F32, tag="ev")
                            oe = getattr(nc, OUT_EVAC_ENG)
                            if oe is nc.scalar:
                                oe.copy(ev[:, 0:iw], op_ps[:, 0:iw])
                            else:
                                oe.tensor_copy(ev[:, 0:iw], op_ps[:, 0:iw])
                            nc.sync.dma_start(
                                out_view[:, cc, i0 : i0 + iw], ev[:, 0:iw]
                            )

                    p_queue.append((gc_unit, outproj))

                c_queue.append((gctr[0] - 1, closing))

            for _rep in range(reps):
                proj_batch(0)
                for ib in range(len(IBLOCKS)):
                    mega = load_bias(ib)
                    for b in range(B):
                        if ib == 0 and b >= 1:
                            proj_batch(b)
                        attn(b, ib, mega, ASSIGN[ib])
                flush_all()
    return nc


_CACHE = {}


def _build(reps=1):
    key = ("nc", reps, ASSIGN, LAG_E, LAG_O, LAG_C, LAG_P, PP_BUFS,
           SPSUM_BUFS, OUT_EVAC_ENG, QHI_EVAC_ENG, QLO_EVAC_ENG)
    if key not in _CACHE:
        nc = bacc.Bacc("TRN2", target_bir_lowering=False, debug=False,
                       num_devices=HEADS)
        _emit(nc, reps=reps)
        nc.compile()
        _CACHE[key] = nc
    return _CACHE[key]


def _prep_inputs(x, pos_bias, w_qkv, w_out):
    bf16 = ml_dtypes.bfloat16
    xf = np.ascontiguousarray(x.reshape(B, C, N).astype(bf16))
    rt = float(np.sqrt(A16))
    ident = np.eye(128, dtype=np.float32).astype(E4NP)
    in_maps = []
    for h in range(HEADS):
        wq = w_qkv[h * D : (h + 1) * D, :].T * np.float32(SCALE * rt)
        wk = w_qkv[C + h * D : C + (h + 1) * D, :].T * np.float32(rt)
        wv = np.ascontiguousarray(w_qkv[2 * C + h * D : 2 * C + (h + 1) * D, :].T)
        wqk = np.concatenate([wq, wk], axis=1)  # [C, 64]
        wo = np.ascontiguousarray(w_out[:, h * D : (h + 1) * D].T)  # [32, 256]
        wo2 = np.zeros((97, C), dtype=np.float32)
        wo2[0:D] = wo
        wo2[64 : 64 + D] = wo
        bt = np.ascontiguousarray(pos_bias[h].T * np.float32(A16)).astype(E4NP)
        in_maps.append(
            {
                "x": xf,
                "wqk": np.ascontiguousarray(wqk).astype(bf16),
                "wv": wv.astype(bf16),
                "wo": wo2,
                "bt": bt,
                "ident": ident,
            }
        )
    return in_maps


def _run(inputs, trace=False):
    x = np.asarray(inputs["x"], dtype=np.float32)
    pos_bias = np.asarray(inputs["pos_bias"], dtype=np.float32)
    w_qkv = np.asarray(inputs["w_qkv"], dtype=np.float32)
    w_out = np.asarray(inputs["w_out"], dtype=np.float32)
    b_out = np.asarray(inputs["b_out"], dtype=np.float32)

    nc = _build()
    in_maps = _prep_inputs(x, pos_bias, w_qkv, w_out)
    res = bass_utils.run_bass_kernel_spmd(
        nc, in_maps, core_ids=list(range(HEADS)), trace=trace
    )
    out = np.zeros((B, C, N), dtype=np.float32)
    for h in range(HEADS):
        o = res.results[h]["out_un"]
        s = res.results[h]["sums"]
        out += o / (s[:, 0][:, None, :] + s[:, 1][:, None, :])
    out += b_out[None, :, None]
    return out.reshape(B, C, H, W).astype(np.float32), res


def kernel(**inputs):
    return _run(inputs)[0]


# revision 6
# speedup vs baseline: 2.9394x; 2.9394x over previous
"""Multi-head attention kernel for Trainium2, one head per NeuronCore.

Math per head h, batch b (n = 2304 tokens, c = 256, d = 32):
  q,k = W_{q,k} @ x   (joint matmul, bf16), v = x^T @ Wv^T (transposed)
  S''[j,i] = A16*(k_j . q_i * scale + B[j,i])   computed in fp8 DoubleRow
    matmuls with the A16 = 128/ln2 Schraudolph scale folded into the
    host-prepped weights and bias:
      m2 = {k_lo, q_hi} + {I, B''}   (DR pair, K=128: identity adds bias)
      m1 = {k_hi, q_hi} + {k_hi, q_lo}  (DR pair, K=32, stride-0 lhsT)
    where q = q_hi + q_lo and k = k_hi + k_lo are exact fp8e4m3 hi/lo
    splits, so S'' carries only the tiny -k_lo*q_lo residual.
  P[j,i] (bf16) = exp(S''/A16) via either:
      'A': Act exp with scale=1/A16   (exact)
      'D': DVE Schraudolph: bf16bits(P) = int16(round(S'' + B16))
  O accumulation: v_ext = [v.T | 1] per 128-token chunk, col-tiled 2-way
    (even chunks -> psum rows 0..32, odd -> 64..96).
  out_un[c,i] = sum_r wo2[c,r] * o_t[r,i]  (f32r, wo2 = [wo;0;wo;0])
Host: out = sum_h out_un_h / (sums0_h + sums1_h) + b_out.
"""

import sys

for _p in ("/opt/trn_rl_repo", "/root/.axon_site/_ro/trn_rl_repo"):
    if _p not in sys.path:
        sys.path.append(_p)

import numpy as np
import ml_dtypes

import concourse.bacc as bacc
import concourse.mybir as mybir
import concourse.tile as tile
from concourse import bass_utils
from concourse.ap import AP

HEADS = 8
D = 32
SCALE = D ** -0.5
B = 4
C = 256
N = 2304
H = W = 48
NJ = 18                     # 128-row j-chunks
NG = 9                      # pair-groups per (b, i-block)
IBLOCKS = [(0, 512), (512, 512), (1024, 512), (1536, 512), (2048, 256)]

F32 = mybir.dt.float32
F32R = mybir.dt.float32r
BF16 = mybir.dt.bfloat16
I16 = mybir.dt.int16
I8 = mybir.dt.int8
FP8 = mybir.dt.float8e4
E4NP = ml_dtypes.float8_e4m3fn
EXP = mybir.ActivationFunctionType.Exp
ADD = mybir.AluOpType.add
MAX = mybir.AluOpType.max
DR = mybir.MatmulPerfMode.DoubleRow

A16 = float(128.0 / np.log(2.0))     # Schraudolph scale folded into wq/wk/B
SCH_B16 = 16256.0 - 7.37             # bf16 exp bias, mean-centered (RN conv)

# k-region free-layout (fp8): per chunk c: k_hi at 256c, k_lo at 256c+128
# (rows 0..31, rows 32..127 zero); identity I at KREG_I (rows 0..127).
KREG_I = NJ * 256            # 4608
KREG_W = KREG_I + 128        # 4736
# mega rhs tile (per i-block) free-layout (fp8), slot width 512:
# q_hi(b) at b*512, q_lo(b) at (4+b)*512, B-chunk(c) at (8+c)*512.
# q rows 0..31 data, 32..127 zero.  Slot strides stay under the 16-bit
# ISA step_elem limit.
SLOT = 512
MEGA_W = (8 + NJ) * SLOT

# tuning knobs
ASSIGN = (
    "ADADADADA",
    "ADADADADA",
    "ADADADADA",
    "ADADADADA",
    "ADADADADA",
)
LAG_E = 2                  # pair-groups of slack: S psum -> exp
LAG_O = 8                  # exp -> O matmuls
LAG_C = LAG_O              # unit closing (o_t evac + sums)
LAG_P = LAG_O + 4          # out-projection
PP_BUFS = 14
SPSUM_BUFS = 3
OUT_EVAC_ENG = "scalar"
QHI_EVAC_ENG = "scalar"
QLO_EVAC_ENG = "vector"


def _mk_ap(base_ap, extra_off, dims):
    return AP(base_ap.tensor, base_ap.offset + extra_off, dims)


def _emit(nc, reps=1):
    x_d = nc.dram_tensor("x", [B, C, N], BF16, kind="ExternalInput")
    wqk_d = nc.dram_tensor("wqk", [C, 64], BF16, kind="ExternalInput")
    wv_d = nc.dram_tensor("wv", [C, D], BF16, kind="ExternalInput")
    wo_d = nc.dram_tensor("wo", [97, C], F32, kind="ExternalInput")
    # int8-declared (bitcast to fp8 on device): fp8 dtypes cannot cross the
    # PJRT transfer path on this backend
    bt_d = nc.dram_tensor("bt", [N, N], I8, kind="ExternalInput")
    id_d = nc.dram_tensor("ident", [128, 128], I8, kind="ExternalInput")
    out_d = nc.dram_tensor("out_un", [B, C, N], F32, kind="ExternalOutput")
    sums_d = nc.dram_tensor("sums", [B, 2, N], F32, kind="ExternalOutput")

    with tile.TileContext(nc) as tc:
        with (
            tc.tile_pool(name="wpool", bufs=1) as wpool,
            tc.tile_pool(name="big", bufs=2) as bigpool,
            tc.tile_pool(name="kreg", bufs=4) as kpool,
            tc.tile_pool(name="ktmp", bufs=2) as ktpool,
            tc.tile_pool(name="mega", bufs=2) as megapool,
            tc.tile_pool(name="qst", bufs=4) as qstpool,
            tc.tile_pool(name="vext", bufs=4) as vpool,
            tc.tile_pool(name="pp", bufs=PP_BUFS) as ppool,
            tc.tile_pool(name="outsb", bufs=3) as outpool,
            tc.tile_pool(name="osb", bufs=2) as opool,
            tc.tile_pool(name="spsum", bufs=SPSUM_BUFS, space="PSUM") as spsum,
            tc.tile_pool(name="psO", bufs=1, space="PSUM") as psO,
            tc.tile_pool(name="psP", bufs=1, space="PSUM") as psP,
        ):
            # ---- static weights ----
            wqk_r = wpool.tile([128, 2, 64], BF16, tag="wqk")
            nc.sync.dma_start(wqk_r, wqk_d.ap().rearrange("(cc p) m -> p cc m", p=128))
            wv_r = wpool.tile([128, 2, D], BF16, tag="wv")
            nc.sync.dma_start(wv_r, wv_d.ap().rearrange("(cc p) m -> p cc m", p=128))
            wo_raw = wpool.tile([97, C], F32, tag="woraw")
            nc.sync.dma_start(wo_raw, wo_d.ap())
            wo_r = wpool.tile([97, C], F32R, tag="wor")
            nc.vector.tensor_copy(wo_r, wo_raw)

            # mega rhs tile: q slots + B chunks; zero q-slot pad rows once
            # pre-zero the q-slot pad rows of both rotating mega buffers
            for _ in range(2):
                mz = megapool.tile([128, MEGA_W], FP8, tag="mega")
                nc.vector.memset(mz[32:64, 0 : 8 * SLOT], 0.0)
                nc.vector.memset(mz[64:128, 0 : 8 * SLOT], 0.0)
            # k-region per batch: zero pad rows once, load identity
            kregs = []
            for b in range(B):
                kr = kpool.tile([128, KREG_W], FP8, tag="kreg")
                kregs.append(kr)
                nc.vector.memset(kr[32:64, 0:KREG_I], 0.0)
                nc.vector.memset(kr[64:128, 0:KREG_I], 0.0)
                nc.sync.dma_start(kr.bitcast(I8)[:, KREG_I:KREG_W], id_d.ap())

            v_sb = [None] * B
            q_st = [None] * B

            def load_bias(ib):
                i0, iw = IBLOCKS[ib]
                mega = megapool.tile([128, MEGA_W], FP8, tag="mega")
                for c in range(NJ):
                    nc.sync.dma_start(
                        mega.bitcast(I8)[:, (8 + c) * SLOT : (8 + c) * SLOT + iw],
                        bt_d.ap().rearrange("(c p) i -> p c i", p=128)[
                            :, c, i0 : i0 + iw
                        ],
                    )
                return mega

            def proj_batch(b):
                x_r = bigpool.tile([128, 2, N], BF16, tag="big")
                x_view = x_d.ap()[b].rearrange("(cc p) n -> p cc n", p=128)
                for cc in range(2):
                    nc.sync.dma_start(x_r[:, cc, :], x_view[:, cc, :])

                ktmp = ktpool.tile([64, KREG_I], FP8, tag="ktmp")
                qst = qstpool.tile([32, 2 * N], FP8, tag="qst")
                q_st[b] = qst
                qhi_eng = getattr(nc, QHI_EVAC_ENG)
                qlo_eng = getattr(nc, QLO_EVAC_ENG)
                for ib, (i0, iw) in enumerate(IBLOCKS):
                    pt = spsum.tile([128, 1024], F32, tag="sg")
                    for cc in range(2):
                        nc.tensor.matmul(
                            pt[0:64, 0:iw],
                            wqk_r[:, cc, :],
                            x_r[:, cc, i0 : i0 + iw],
                            start=(cc == 0),
                            stop=(cc == 1),
                        )
                    # evacs: q rows 0..31 -> staging; k rows 32..63 -> ktmp
                    qh_dst = qst[:, i0 : i0 + iw]
                    ql_dst = qst[:, N + i0 : N + i0 + iw]
                    if qhi_eng is nc.scalar:
                        qhi_eng.copy(qh_dst, pt[0:32, 0:iw])
                    else:
                        qhi_eng.tensor_copy(qh_dst, pt[0:32, 0:iw])
                    qlo_eng.tensor_sub(ql_dst, pt[0:32, 0:iw], qh_dst)
                    # k chunk slots covered by this i-block
                    nc4 = iw // 128
                    kh_dst = _mk_ap(
                        ktmp[32:64, 0:1], i0 * 2,
                        [list(ktmp[32:64, 0:1].ap[0]), [256, nc4], [1, 128]],
                    )
                    kl_dst = _mk_ap(
                        ktmp[32:64, 0:1], i0 * 2 + 128,
                        [list(ktmp[32:64, 0:1].ap[0]), [256, nc4], [1, 128]],
                    )
                    kh_src = pt[32:64, 0:iw].rearrange("p (c j) -> p c j", j=128)
                    if qhi_eng is nc.scalar:
                        qhi_eng.copy(kh_dst, kh_src)
                    else:
                        qhi_eng.tensor_copy(kh_dst, kh_src)
                    qlo_eng.tensor_sub(kl_dst, kh_src, kh_dst)
                # partition-shift k rows 32..63 -> kreg rows 0..31
                nc.sync.dma_start(kregs[b][0:32, 0:KREG_I], ktmp[32:64, :])

                # v transposed: v_T[j, d] per chunk, with ones column
                vext = vpool.tile([128, NJ * (D + 1)], BF16, tag="vext")
                v_sb[b] = vext
                nc.vector.memset(vext, 1.0)
                vt = spsum.tile([128, 1024], F32, tag="sg")
                for jc in range(NJ):
                    for cc in range(2):
                        nc.tensor.matmul(
                            vt[:, jc * D : (jc + 1) * D],
                            x_r[:, cc, jc * 128 : (jc + 1) * 128],
                            wv_r[:, cc, :],
                            start=(cc == 0),
                            stop=(cc == 1),
                        )
                nc.vector.tensor_copy(
                    vext.rearrange("p (jc m) -> p jc m", m=D + 1)[:, :, 0:D],
                    vt.rearrange("p (jc m) -> p jc m", m=D)[:, 0:NJ, :],
                )

            # Deferred-emission queues (see baseline): strict FIFO engine
            # queues mean a dependent instruction emitted too early
            # head-of-line-blocks its engine.  Stage queues with lags:
            #   e_queue: exp ops (wait on S psum)
            #   o_queue: O matmuls (wait on P)
            #   c_queue: per-(b,ib) o_t evac + sums DMA
            #   p_queue: per-(b,ib) out-projection
            e_queue, o_queue, c_queue, p_queue = [], [], [], []
            gctr = [0]
            lagged = ((e_queue, LAG_E), (o_queue, LAG_O),
                      (c_queue, LAG_C), (p_queue, LAG_P))

            def pump(cur):
                for q, lag in lagged:
                    while q and q[0][0] <= cur - lag:
                        q.pop(0)[1]()

            def flush_all():
                while any(q for q, _ in lagged):
                    cands = [
                        (q[0][0], i, q) for i, (q, _) in enumerate(lagged) if q
                    ]
                    cands.sort()
                    cands[0][2].pop(0)[1]()

            def attn(b, ib, mega, eb_path):
                i0, iw = IBLOCKS[ib]
                kreg = kregs[b]
                # stage this batch's q hi/lo into the i-block mega slots
                nc.sync.dma_start(
                    mega[0:32, b * SLOT : b * SLOT + iw],
                    q_st[b][:, i0 : i0 + iw],
                )
                nc.sync.dma_start(
                    mega[0:32, (4 + b) * SLOT : (4 + b) * SLOT + iw],
                    q_st[b][:, N + i0 : N + i0 + iw],
                )
                o_ps_box = []

                def get_o_ps():
                    if not o_ps_box:
                        o_ps_box.append(
                            psO.tile([128, 512], F32, tag="po", name="o_ps")
                        )
                    return o_ps_box[0]

                for g in range(NG):
                    path = eb_path[g]
                    s_ps = spsum.tile([128, 1024], F32, tag="sg")
                    for cz in range(2):
                        c = 2 * g + cz
                        off = cz * iw
                        # m2: {k_lo, q_hi} + {I, B''}  (K=128 DR)
                        lhsT2 = _mk_ap(
                            kreg[:, 0:1], 256 * c + 128,
                            [list(kreg[:, 0:1].ap[0]),
                             [KREG_I - (256 * c + 128), 2], [1, 128]],
                        )
                        rhs2 = _mk_ap(
                            mega[:, 0:1], b * SLOT,
                            [list(mega[:, 0:1].ap[0]),
                             [(8 + c - b) * SLOT, 2], [1, iw]],
                        )
                        nc.tensor.matmul(
                            s_ps[:, off : off + iw], lhsT2, rhs2,
                            start=True, stop=False, perf_mode=DR,
                        )
                        # m1: {k_hi, q_hi} + {k_hi, q_lo}  (K=32 DR)
                        lhsT1 = _mk_ap(
                            kreg[0:32, 0:1], 256 * c,
                            [list(kreg[0:32, 0:1].ap[0]), [0, 2], [1, 128]],
                        )
                        rhs1 = _mk_ap(
                            mega[0:32, 0:1], b * SLOT,
                            [list(mega[0:32, 0:1].ap[0]),
                             [4 * SLOT, 2], [1, iw]],
                        )
                        nc.tensor.matmul(
                            s_ps[:, off : off + iw], lhsT1, rhs1,
                            start=False, stop=True, perf_mode=DR,
                        )

                    p_t = ppool.tile([128, 1024], BF16, tag="pt")
                    gc = gctr[0]
                    gctr[0] += 1

                    def e_thunk(path=path, p_t=p_t, s_ps=s_ps, iw=iw):
                        if path == "A":
                            nc.scalar.activation(
                                p_t[:, 0 : 2 * iw], s_ps[:, 0 : 2 * iw], EXP,
                                bias=0.0, scale=float(1.0 / A16),
                            )
                        else:
                            nc.vector.tensor_scalar(
                                p_t.bitcast(I16)[:, 0 : 2 * iw],
                                s_ps[:, 0 : 2 * iw],
                                float(SCH_B16), 0.0, ADD, MAX,
                            )

                    e_queue.append((gc, e_thunk))

                    def o_thunk(g=g, p_t=p_t, b=b, iw=iw):
                        o_ps = get_o_ps()
                        for cz in range(2):
                            c = 2 * g + cz
                            base = 64 * (c % 2)
                            nc.tensor.matmul(
                                o_ps[base : base + D + 1, 0:iw],
                                v_sb[b][:, c * (D + 1) : (c + 1) * (D + 1)],
                                p_t[:, cz * iw : (cz + 1) * iw],
                                start=(c < 2),
                                stop=(c >= NJ - 2),
                            )

                    o_queue.append((gc, o_thunk))
                    pump(gc)

                def closing(b=b, i0=i0, iw=iw, gc_unit=gctr[0] - 1):
                    o_ps = get_o_ps()
                    o_t = opool.tile([128, 512], F32R, tag="ot")
                    # single evac covers both col-tile bands; rows 33..63
                    # carry stale psum but wo2 zeros there kill them.
                    nc.scalar.copy(o_t[0:97, 0:iw], o_ps[0:97, 0:iw])
                    nc.sync.dma_start(
                        sums_d.ap()[b, 0, i0 : i0 + iw],
                        o_t[D : D + 1, 0:iw].bitcast(F32),
                    )
                    nc.sync.dma_start(
                        sums_d.ap()[b, 1, i0 : i0 + iw],
                        o_t[96:97, 0:iw].bitcast(F32),
                    )

                    def outproj(b=b, i0=i0, iw=iw, o_t=o_t):
                        out_view = out_d.ap()[b].rearrange(
                            "(cc p) n -> p cc n", p=128
                        )
                        for cc in range(2):
                            op_ps = psP.tile(
                                [128, 512], F32, tag="pp", name="op_ps"
                            )
                            nc.tensor.matmul(
                                op_ps[:, 0:iw],
                                wo_r[0:97, cc * 128 : (cc + 1) * 128],
                                o_t[0:97, 0:iw],
                                start=True,
                                stop=True,
                            )
                            ev = outpool.tile([128, 512], F32, tag="ev")
                            oe = getattr(nc, OUT_EVAC_ENG)
                            if oe is nc.scalar:
                                oe.copy(ev[:, 0:iw], op_ps[:, 0:iw])
                            else:
                                oe.tensor_copy(ev[:, 0:iw], op_ps[:, 0:iw])
                            nc.sync.dma_start(
                                out_view[:, cc, i0 : i0 + iw], ev[:, 0:iw]
                            )

                    p_queue.append((gc_unit, outproj))

                c_queue.append((gctr[0] - 1, closing))

            for _rep in range(reps):
                proj_batch(0)
                for ib in range(len(IBLOCKS)):
                    mega = load_bias(ib)
                    for b in range(B):
                        if ib == 0 and b >= 1:
                            proj_batch(b)
                        attn(b, ib, mega, ASSIGN[ib])
                flush_all()
    return nc


_CACHE = {}


def _build(reps=1):
    key = ("nc", reps, ASSIGN, LAG_E, LAG_O, LAG_C, LAG_P, PP_BUFS,
           SPSUM_BUFS, OUT_EVAC_ENG, QHI_EVAC_ENG, QLO_EVAC_ENG)
    if key not in _CACHE:
        nc = bacc.Bacc("TRN2", target_bir_lowering=False, debug=False,
                       num_devices=HEADS)
        _emit(nc, reps=reps)
        nc.compile()
        _CACHE[key] = nc
    return _CACHE[key]


def _prep_inputs(x, pos_bias, w_qkv, w_out):
    bf16 = ml_dtypes.bfloat16
    xf = np.ascontiguousarray(x.reshape(B, C, N).astype(bf16))
    rt = float(np.sqrt(A16))
    ident = np.eye(128, dtype=np.float32).astype(E4NP).view(np.int8)
    in_maps = []
    for h in range(HEADS):
        wq = w_qkv[h * D : (h + 1) * D, :].T * np.float32(SCALE * rt)
        wk = w_qkv[C + h * D : C + (h + 1) * D, :].T * np.float32(rt)
        wv = np.ascontiguousarray(w_qkv[2 * C + h * D : 2 * C + (h + 1) * D, :].T)
        wqk = np.concatenate([wq, wk], axis=1)  # [C, 64]
        wo = np.ascontiguousarray(w_out[:, h * D : (h + 1) * D].T)  # [32, 256]
        wo2 = np.zeros((97, C), dtype=np.float32)
        wo2[0:D] = wo
        wo2[64 : 64 + D] = wo
        bt = np.ascontiguousarray(pos_bias[h].T * np.float32(A16)).astype(E4NP).view(np.int8)
        in_maps.append(
            {
                "x": xf,
                "wqk": np.ascontiguousarray(wqk).astype(bf16),
                "wv": wv.astype(bf16),
                "wo": wo2,
                "bt": bt,
                "ident": ident,
            }
        )
    return in_maps


def _run(inputs, trace=False):
    x = np.asarray(inputs["x"], dtype=np.float32)
    pos_bias = np.asarray(inputs["pos_bias"], dtype=np.float32)
    w_qkv = np.asarray(inputs["w_qkv"], dtype=np.float32)
    w_out = np.asarray(inputs["w_out"], dtype=np.float32)
    b_out = np.asarray(inputs["b_out"], dtype=np.float32)

    nc = _build()
    in_maps = _prep_inputs(x, pos_bias, w_qkv, w_out)
    res = bass_utils.run_bass_kernel_spmd(
        nc, in_maps, core_ids=list(range(HEADS)), trace=trace
    )
    out = np.zeros((B, C, N), dtype=np.float32)
    for h in range(HEADS):
        o = res.results[h]["out_un"]
        s = res.results[h]["sums"]
        out += o / (s[:, 0][:, None, :] + s[:, 1][:, None, :])
    out += b_out[None, :, None]
    return out.reshape(B, C, H, W).astype(np.float32), res


def kernel(**inputs):
    return _run(inputs)[0]
